# revision 35
# baseline (speedup 1.0000x reference)
"""Sparse (diffusion block-causal) GQA attention on 8 Trainium2 NeuronCores.

Contract: kernel(**inputs) takes the FULL inputs
    q [2048, 4096] f32, k [2048, 1024] f32, v [2048, 1024] f32,
    block_mask [2048, 2048] bool
and returns the FULL output [2048, 4096] f32.

Sharding: tensor-parallel over KV heads. Core c owns KV head c and its 4
GQA query heads (output columns [512c, 512c+512)). No inter-core
communication.

Device algorithm per core (S^T layout [k partitions, q free]):
  Work = 16 (head, q-chunk) pairs whose score tiles are flattened into
  ONE globally packed stream of "rounds": each round packs 1536 columns
  of score tiles (full 512-wide k-tiles + the diagonal partial tiles at
  their active widths 512/384/128/256), first-fit so no tile crosses a
  PSUM bank and no gap cells exist. 46 uniform rounds total, exactly the
  active-score width (69632 cols) — zero wasted exp columns. PSUM: score
  tile [128,1536] f32 = 3 banks, double buffered (6) + 2 po banks.
    QK^T: fp16 matmuls (1 cycle/col at any width, unlike f32r which is
      4x slower below 256 cols).
    diagonal mask: one shared [128,128] bf16 additive -1e30 pattern via
      an identity-matmul accumulate (the 32-block staircase is identical
      for every diagonal tile), folded into the score PSUM group.
    exp on ACT: ONE activation per round over the packed [0:used] range
      (46 calls; measured HW rate 1.142 ns/elem from PSUM — the ACT
      engine is the binding resource, ~84 us/core floor).
    PV: po[d, q] += V_j^T @ es slice (PSUM accum over the chunk).
    denominators: fp16 accumulate per chunk on DVE (2x mode, 355 ns per
      add on HW); the final [128, 512] partial-sum tile is DMA'd out
      and reduced on the HOST (kills the ones-matmuls and a PSUM bank).
  Epilogue per chunk: the denominator partials and a DVE-staged fp16
  copy of po (DMA cannot read PSUM) share ONE [128, 1024] tile and go
  out in ONE DMA (halves issue slots + completion semaphores, shortens
  the final-chunk drain). Host: split, reduce, transpose, divide.

Measured dead ends (kept as off-by-default flags): offloading exp rounds
to DVE via 1-op Schraudolph (DVE_EVERY) — DVE runs at the same 1.137
ns/elem and queue interference costs more than ACT saves; a second
denominator chain on the Pool engine — Pool adds are 1.18 us and wreck
the pipeline; For_i staggered_reset — stage transitions cost more than
the per-iteration barrier (~6.6 us) they replace.

The activation table load is hoisted out of the reps loop via a dummy
pre-loop exp.
"""

import os
import sys

import numpy as np

for _p in ("/opt/trn_rl_repo",):
    if _p not in sys.path and os.path.isdir(_p):
        sys.path.insert(0, _p)

S = 2048
H = 32
HKV = 8
G = H // HKV  # 4 query heads per kv head
D = 128
NCORES = 8
SCALE = float(D) ** -0.5
CHUNK = 512  # q columns per chunk
KT = 128  # k rows per tile (PE partition dim)
ROUND_W = 1536  # packed exp-round width (3 PSUM banks)
BANK_W = 512  # f32 columns per PSUM bank
PATW = 128  # mask pattern window width
NEG = -1.0e30

PS_BUFS = 2
PO_BUFS = 2
ES_BUFS = 6
ACC_BUFS = 6
CHUNK_ORDER = "byJ"  # "byJ" or "byH"
# For_i(staggered_reset=True) drops the per-iteration all-engine barrier
# + semaphore-reset block (~6.6 us/rep measured) in the reps-loop used
# for timing; the body is split into 4 semaphore stages instead.
STAGGERED = False  # measured: staggered stage transitions cost more
                   # than the single barrier they replace (92.6 vs 90.4 us)
# Hybrid exp: every DVE_EVERY-th all-full round computes exp on DVE via a
# one-instruction Schraudolph approximation (y = bitcast_f16(int16(A*x+B)),
# ~3% max rel err on those tiles, round-to-nearest verified on HW) to
# offload the saturated ACT engine. 0 disables.
DVE_EVERY = 0
LOG2E = 1.4426950408889634
SCH_A = 1024.0 * LOG2E  # multiplied by SCALE at emission
SCH_B = 15360.0 - 44.4  # balanced two-sided error

NJ = S // CHUNK  # q chunks
NK = S // KT  # k tiles

_program_cache = {}
last_exec_time_ns = None
last_results = None


def _schedule_from_mask(bm):
    """Classify each (q-chunk J, k-tile j) as full / empty / partial and
    pack each chunk's tiles into exp rounds.

    Returns (cache_key, sched, patterns): sched[J] is a list of rounds,
    each round a (tiles, used) pair with tiles = [(j, q0, pat_idx, off)].
    patterns is a list of [KT, PATW] f32 additive-mask windows (0 where
    attending, NEG where masked), k-major. Partial tiles must have all
    cells active outside the window rows [q0, q0+PATW) (holds for the
    diffusion block-causal mask).
    """
    patterns = []
    pat_rects = []
    pat_idx = {}
    per_J = []  # per q-chunk: ordered tile list [(j, q0, pat_idx)]
    for J in range(NJ):
        rows = bm[J * CHUNK : (J + 1) * CHUNK]  # [CHUNK q, S k]
        fulls = []
        parts = []
        for j in range(NK):
            sub = rows[:, j * KT : (j + 1) * KT]  # [q, k]
            if sub.all():
                fulls.append((j, 0, None))
            elif not sub.any():
                continue
            else:
                q0 = int(np.argmax(sub.any(axis=1)))
                w = CHUNK - q0
                pw = min(PATW, w)
                if q0 + pw < CHUNK:
                    assert sub[q0 + pw :].all(), (
                        "mask cells outside the 128-row window are not all "
                        "active; unsupported mask structure"
                    )
                win = sub[q0 : q0 + pw]  # [pw, KT]
                key = win.tobytes()
                if key not in pat_idx:
                    pat_idx[key] = len(patterns)
                    pat = np.zeros((KT, PATW), np.float32)
                    pat[:, :pw] = np.where(
                        win.T, np.float32(0.0), np.float32(NEG)
                    )
                    # Masked cells as per-column k-suffix rectangles (cols
                    # grouped by equal suffix start): lets the kernel zero
                    # the es staircase with a few Pool-engine memsets
                    # instead of a mask-add matmul on PE. None if the
                    # masked set is not suffix-form (fallback: matmul).
                    rects = []
                    ok = True
                    winT = win.T  # [KT, pw] k-major
                    starts = np.full(pw, KT, np.int64)
                    for c in range(pw):
                        col = winT[:, c]
                        n_act = int(col.sum())
                        if not col[:n_act].all():
                            ok = False
                            break
                        starts[c] = n_act
                    if ok:
                        c = 0
                        while c < pw:
                            c2 = c
                            while c2 < pw and starts[c2] == starts[c]:
                                c2 += 1
                            if starts[c] < KT:
                                rects.append((c, c2, int(starts[c])))
                            c = c2
                    patterns.append(pat)
                    pat_rects.append(rects if ok else None)
                parts.append((j, q0, pat_idx[key]))
        assert fulls or parts, f"q-chunk {J} attends to nothing"
        parts.sort(key=lambda t: t[1])  # widest first
        tiles = fulls + parts
        assert tiles[0][1] == 0, "chunk needs a q0 == 0 tile first"
        per_J.append(tiles)

    # Global packing: flatten all (h, J) chunks (by-J phases) into one
    # tile stream and first-fit into uniform ROUND_W rounds such that no
    # tile crosses a PSUM bank and no gaps form (gap cells would be
    # exp'd stale PSUM). A lookahead of one chunk fills bank remainders
    # at chunk boundaries. A chunk's first placed tile must be its
    # q0 == 0 tile (PV/acc accumulation start covers the full q range).
    order = [(h, J) for J in sorted(range(NJ), reverse=True) for h in range(G)]
    queues = [
        [(h, J, j, q0, p) for (j, q0, p) in per_J[J]] for h, J in order
    ]
    rounds = []  # [( [(h,J,j,q0,pidx,off)...], used )]
    live = []  # queue indices started & unfinished (max 2: po banks)
    nexti = 0
    cur = []
    off = 0
    while live or nexti < len(queues):
        rem = min(ROUND_W - off, BANK_W - (off % BANK_W))
        cands = [
            (qi, t) for qi in live for t in queues[qi] if CHUNK - t[3] <= rem
        ]
        if len(live) < 2 and nexti < len(queues):
            t0 = queues[nexti][0]  # a chunk opens with its q0==0 tile
            if CHUNK - t0[3] <= rem:
                cands.append((nexti, t0))
        if not cands:
            assert cur, "packing deadlock"
            rounds.append((cur, off))
            cur = []
            off = 0
            continue
        # widest first; tie-break toward the oldest chunk (drain early)
        qi, t = min(cands, key=lambda c: (-(CHUNK - c[1][3]), c[0]))
        h, J, j, q0, p = t
        queues[qi].remove(t)
        if qi == nexti:
            live.append(qi)
            nexti += 1
        if not queues[qi]:
            live.remove(qi)
        cur.append((h, J, j, q0, p, off))
        off += CHUNK - q0
    if cur:
        rounds.append((cur, off))
    ntiles = {}
    for tiles, _ in rounds:
        for h, J, j, q0, p, off in tiles:
            ntiles[(h, J)] = ntiles.get((h, J), 0) + 1
    sched = (rounds, ntiles, pat_rects)
    cache_key = (
        tuple(tuple(tuple(t) for t in r) + (u,) for r, u in rounds),
        tuple(p.tobytes() for p in patterns),
        tuple(tuple(r) if r is not None else None for r in pat_rects),
    )
    return hash(cache_key), sched, patterns


def _build_program(sched, patterns, reps=1, unroll=1):
    import contextlib

    import concourse.bacc as bacc
    import concourse.tile as tile
    from concourse import mybir

    f32 = mybir.dt.float32
    f16 = mybir.dt.float16
    bf16 = mybir.dt.bfloat16
    EXP = mybir.ActivationFunctionType.Exp

    nc = bacc.Bacc(
        "TRN2", target_bir_lowering=False, debug=False, num_devices=NCORES
    )

    qT = nc.dram_tensor("qT", [G, D, S], f16, kind="ExternalInput").ap()
    kT = nc.dram_tensor("kT", [D, S], f16, kind="ExternalInput").ap()
    v = nc.dram_tensor("v", [S, D], f16, kind="ExternalInput").ap()
    n_pat = max(1, len(patterns))
    pmask = nc.dram_tensor(
        "pmask", [n_pat, KT, PATW], bf16, kind="ExternalInput"
    ).ap()
    ident = nc.dram_tensor("ident", [D, D], bf16, kind="ExternalInput").ap()
    # Merged per-chunk epilogue output: [:, :CHUNK] = denominator
    # partial sums (k-partition rows), [:, CHUNK:] = O^T chunk (d rows).
    # One DMA per chunk instead of two (fewer issue slots + completion
    # semaphores; shorter drain on the final chunk).
    ep_d = nc.dram_tensor(
        "ep_d", [G * NJ, KT, 2 * CHUNK], f16, kind="ExternalOutput"
    ).ap()

    with tile.TileContext(nc) as tc:
        with (
            tc.tile_pool(name="singles", bufs=1) as singles,
            tc.tile_pool(name="ps", bufs=PS_BUFS, space="PSUM") as ps_pool,
            tc.tile_pool(name="po", bufs=PO_BUFS, space="PSUM") as po_pool,
            tc.tile_pool(name="es", bufs=ES_BUFS) as es_pool,
            tc.tile_pool(name="accp", bufs=ACC_BUFS) as acc_pool,
        ):
            qT_sb = singles.tile([D, G * S], f16)
            kT_sb = singles.tile([D, S], f16)
            v_sb = singles.tile([KT, NK * D], f16)
            pm_sb = singles.tile([KT, n_pat * PATW], bf16)
            id_sb = singles.tile([D, D], bf16)
            dummy = singles.tile([1, 1], f32)

            # Input DMAs, ordered for the startup critical path. Chunk
            # order is by-J phases (h0..h3 at J3, then J2, J1, J0), so
            # load kT's first tiles + everyone's J3 q-slices first, then
            # the bulk, then the later q phases.
            # The two DMAs gating the first QK round issue on different
            # HWDGE queues (SP + Activation) so their ~0.65 us issue
            # latencies overlap; ACT is otherwise idle at startup.
            nc.sync.dma_start(out=kT_sb[:, 0:CHUNK], in_=kT[:, 0:CHUNK])
            nc.scalar.dma_start(
                out=qT_sb[:, 3 * CHUNK : 4 * CHUNK],
                in_=qT[0][:, 3 * CHUNK : 4 * CHUNK],
            )
            nc.sync.dma_start(out=kT_sb[:, CHUNK:], in_=kT[:, CHUNK:])
            nc.sync.dma_start(
                out=qT_sb[:, S:].rearrange(
                    "p (h s) -> p h s", s=S
                )[:, :, 3 * CHUNK : 4 * CHUNK],
                in_=qT[1:].rearrange("h p s -> p h s")[
                    :, :, 3 * CHUNK : 4 * CHUNK
                ],
            )
            nc.sync.dma_start(
                out=v_sb.rearrange("p (t d) -> p t d", d=D),
                in_=v.rearrange("(t p) d -> p t d", p=KT),
            )
            nc.sync.dma_start(
                out=pm_sb.rearrange("p (n c) -> p n c", c=PATW),
                in_=pmask.rearrange("n p c -> p n c"),
            )
            nc.sync.dma_start(out=id_sb, in_=ident)
            for Jc in (2, 1, 0):
                nc.sync.dma_start(
                    out=qT_sb.rearrange("p (h s) -> p h s", s=S)[
                        :, :, Jc * CHUNK : (Jc + 1) * CHUNK
                    ],
                    in_=qT.rearrange("h p s -> p h s")[
                        :, :, Jc * CHUNK : (Jc + 1) * CHUNK
                    ],
                )

            # Hoist the activation-table load out of the reps loop.
            nc.vector.memset(dummy, 0.0)
            nc.scalar.activation(dummy, dummy, EXP, scale=1.0)

            rep_ctx = (
                tc.For_i(0, reps, 1, staggered_reset=STAGGERED)
                if reps > 1
                else contextlib.nullcontext()
            )

            rounds, ntiles, pat_rects = sched

            def emit_pv(prev, ctxs, final=False):
                tiles, es, used = prev
                for h, J, j, q0, pidx, off in tiles:
                    ctx = ctxs[(h, J)]
                    po = ctx["po"]
                    w = CHUNK - q0
                    sl = es[:, off : off + w]
                    first = ctx["done"] == 0
                    last = ctx["done"] == ctx["ntiles"] - 1
                    nc.tensor.matmul(
                        po[:, q0:],
                        lhsT=v_sb[:, j * D : (j + 1) * D],
                        rhs=sl,
                        start=first,
                        stop=last,
                    )
                    # Denominator partial sums on DVE (fp16 2x mode) into
                    # the left half of the merged epilogue tile; the right
                    # half receives the O^T copy. Host splits and reduces.
                    acc = ctx["ep"][:, :CHUNK]
                    if first:
                        nc.vector.tensor_copy(acc, sl)
                    else:
                        nc.vector.tensor_add(acc[:, q0:], acc[:, q0:], sl)
                    ctx["done"] += 1
                    if ctx["done"] == ctx["ntiles"]:
                        ci = h * NJ + J
                        if final:
                            # Drain only: ACT is idle after the last exp,
                            # so its Copy runs in parallel with DVE's
                            # final accumulator adds.
                            nc.scalar.copy(ctx["ep"][:, CHUNK:], po)
                        else:
                            nc.vector.tensor_copy(ctx["ep"][:, CHUNK:], po)
                        nc.sync.dma_start(out=ep_d[ci], in_=ctx["ep"])
                        del ctxs[(h, J)]

            eligible = [
                ri
                for ri, (tiles, _) in enumerate(rounds)
                if all(t[4] is None for t in tiles)
            ]
            dve_rounds = (
                set(eligible[DVE_EVERY - 1 :: DVE_EVERY]) if DVE_EVERY else set()
            )
            n_rounds = len(rounds)
            bounds = {
                (n_rounds * (s + 1)) // 4 for s in range(3)
            } if (reps > 1 and STAGGERED) else set()

            with rep_ctx:
                for _ in range(unroll):
                    ctxs = {}
                    prev = None  # (tiles, es, used) awaiting PV emission
                    for ri, (tiles, used) in enumerate(rounds):
                        if ri in bounds:
                            tc.stage_boundary()
                        ps = ps_pool.tile(
                            [KT, ROUND_W], f32, tag="ps", name="ps"
                        )
                        for h, J, j, q0, pidx, off in tiles:
                            if (h, J) not in ctxs:
                                ctxs[(h, J)] = {
                                    "po": po_pool.tile(
                                        [D, CHUNK], f32, tag="po", name="po"
                                    ),
                                    "ep": acc_pool.tile(
                                        [KT, 2 * CHUNK], f16, tag="ep",
                                        name="ep",
                                    ),
                                    "done": 0,
                                    "ntiles": ntiles[(h, J)],
                                }
                            w = CHUNK - q0
                            rhs_q = qT_sb[
                                :,
                                h * S + J * CHUNK + q0 : h * S
                                + (J + 1) * CHUNK,
                            ]
                            nc.tensor.matmul(
                                ps[:, off : off + w],
                                lhsT=kT_sb[:, j * KT : (j + 1) * KT],
                                rhs=rhs_q,
                                start=True,
                                stop=(pidx is None),
                            )
                            if pidx is not None:
                                pw = min(PATW, w)
                                nc.tensor.matmul(
                                    ps[:, off : off + pw],
                                    lhsT=id_sb,
                                    rhs=pm_sb[
                                        :, pidx * PATW : pidx * PATW + pw
                                    ],
                                    start=False,
                                    stop=True,
                                )
                        if prev is not None:
                            emit_pv(prev, ctxs)
                            prev = None
                        es = es_pool.tile(
                            [KT, ROUND_W], f16, tag="es", name="es"
                        )
                        if ri in dve_rounds:
                            nc.vector.tensor_scalar(
                                es[:, :used].bitcast(mybir.dt.int16),
                                ps[:, :used],
                                SCH_A * SCALE,
                                SCH_B,
                                mybir.AluOpType.mult,
                                mybir.AluOpType.add,
                            )
                        else:
                            nc.scalar.activation(
                                es[:, :used], ps[:, :used], EXP, scale=SCALE
                            )
                        prev = (tiles, es, used)
                    emit_pv(prev, ctxs, final=True)
                    prev = None

    # Pin the ACT table set to the one containing Exp so the table-load
    # pass emits exactly one load (hoisted to the pre-loop dummy exp).
    import concourse.bacc as bacc_mod

    orig_tables = bacc_mod.get_activation_tables

    def _only_ln_exp_set(arch):
        return {
            name: (fns if name == "natural_log_exp_and_others" else set())
            for name, fns in orig_tables(arch).items()
        }

    bacc_mod.get_activation_tables = _only_ln_exp_set
    try:
        nc.compile()
    finally:
        bacc_mod.get_activation_tables = orig_tables
    return nc


def _get_program(bm):
    key, sched, patterns = _schedule_from_mask(bm)
    if key not in _program_cache:
        _program_cache[key] = _build_program(sched, patterns)
    return _program_cache[key], patterns


def _shard_inputs(q, k, v, patterns):
    import ml_dtypes

    bf16 = ml_dtypes.bfloat16
    n_pat = max(1, len(patterns))
    if patterns:
        pm = np.ascontiguousarray(np.stack(patterns).astype(bf16))
    else:
        pm = np.zeros((n_pat, KT, PATW), bf16)
    ident = np.eye(D, dtype=bf16)

    q5 = q.reshape(S, HKV, G, D)
    k4 = k.reshape(S, HKV, D)
    v4 = v.reshape(S, HKV, D)
    in_maps = []
    for c in range(NCORES):
        qTc = np.ascontiguousarray(
            q5[:, c].transpose(1, 2, 0).astype(np.float16)
        )  # [G, D, S]
        kTc = np.ascontiguousarray(k4[:, c].T.astype(np.float16))  # [D, S]
        vc = np.ascontiguousarray(v4[:, c].astype(np.float16))  # [S, D]
        in_maps.append(
            {
                "qT": qTc,
                "kT": kTc,
                "v": vc,
                "pmask": pm,
                "ident": ident,
            }
        )
    return in_maps


def kernel(q, k, v, block_mask):
    global last_exec_time_ns, last_results
    q = np.ascontiguousarray(np.asarray(q, dtype=np.float32))
    k = np.ascontiguousarray(np.asarray(k, dtype=np.float32))
    v = np.ascontiguousarray(np.asarray(v, dtype=np.float32))
    bm = np.ascontiguousarray(np.asarray(block_mask)).astype(bool)

    nc, patterns = _get_program(bm)
    in_maps = _shard_inputs(q, k, v, patterns)

    from concourse.bass_utils import run_bass_kernel_spmd

    res = run_bass_kernel_spmd(nc, in_maps, list(range(NCORES)), trace=False)
    last_exec_time_ns = res.exec_time_ns
    last_results = res

    out = np.empty((S, H * D), np.float32)
    for c in range(NCORES):
        ep = res.results[c]["ep_d"]  # [G*NJ, KT, 2*CHUNK] f16
        l = (
            ep[:, :, :CHUNK].astype(np.float32).sum(axis=1)
        ).reshape(G, NJ * CHUNK)  # [G, S]
        oTc = (
            ep[:, :, CHUNK:]
            .astype(np.float32)
            .reshape(G, NJ, D, CHUNK)
            .transpose(0, 2, 1, 3)
            .reshape(G, D, S)
        )
        oTc = oTc / l[:, None, :]
        out[:, c * G * D : (c + 1) * G * D] = (
            oTc.transpose(2, 0, 1).reshape(S, G * D)
        )
    return out


# revision 36
# speedup vs baseline: 1.0108x; 1.0108x over previous
"""Sparse (diffusion block-causal) GQA attention on 8 Trainium2 NeuronCores.

Contract: kernel(**inputs) takes the FULL inputs
    q [2048, 4096] f32, k [2048, 1024] f32, v [2048, 1024] f32,
    block_mask [2048, 2048] bool
and returns the FULL output [2048, 4096] f32.

Sharding: tensor-parallel over KV heads. Core c owns KV head c and its 4
GQA query heads (output columns [512c, 512c+512)). No inter-core
communication.

Device algorithm per core (S^T layout [k partitions, q free]):
  Work = 16 (head, q-chunk) pairs whose score tiles are flattened into
  ONE globally packed stream of "rounds": each round packs 1536 columns
  of score tiles (full 512-wide k-tiles + the diagonal partial tiles at
  their active widths 512/384/128/256), first-fit so no tile crosses a
  PSUM bank and no gap cells exist. 46 uniform rounds total, exactly the
  active-score width (69632 cols) — zero wasted exp columns. PSUM: score
  tile [128,1536] f32 = 3 banks, double buffered (6) + 2 po banks.
    QK^T: fp16 matmuls (1 cycle/col at any width, unlike f32r which is
      4x slower below 256 cols).
    diagonal mask: one shared [128,128] bf16 additive -1e30 pattern via
      an identity-matmul accumulate (the 32-block staircase is identical
      for every diagonal tile), folded into the score PSUM group.
    exp on ACT: ONE activation per round over the packed [0:used] range
      (46 calls; measured HW rate 1.142 ns/elem from PSUM — the ACT
      engine is the binding resource, ~84 us/core floor).
    PV: po[d, q] += V_j^T @ es slice (PSUM accum over the chunk).
    denominators: fp16 accumulate per chunk on DVE (2x mode, 355 ns per
      add on HW); the final [128, 512] partial-sum tile is DMA'd out
      and reduced on the HOST (kills the ones-matmuls and a PSUM bank).
  Epilogue per chunk: the denominator partials and a DVE-staged fp16
  copy of po (DMA cannot read PSUM) share ONE [128, 1024] tile and go
  out in ONE DMA (halves issue slots + completion semaphores, shortens
  the final-chunk drain). Host: split, reduce, transpose, divide.

Measured dead ends (kept as off-by-default flags): offloading exp rounds
to DVE via 1-op Schraudolph (DVE_EVERY) — DVE runs at the same 1.137
ns/elem and queue interference costs more than ACT saves; a second
denominator chain on the Pool engine — Pool adds are 1.18 us and wreck
the pipeline; For_i staggered_reset — stage transitions cost more than
the per-iteration barrier (~6.6 us) they replace.

The activation table load is hoisted out of the reps loop via a dummy
pre-loop exp.
"""

import os
import sys

import numpy as np

for _p in ("/opt/trn_rl_repo",):
    if _p not in sys.path and os.path.isdir(_p):
        sys.path.insert(0, _p)

S = 2048
H = 32
HKV = 8
G = H // HKV  # 4 query heads per kv head
D = 128
NCORES = 8
SCALE = float(D) ** -0.5
CHUNK = 512  # q columns per chunk
KT = 128  # k rows per tile (PE partition dim)
ROUND_W = 1536  # packed exp-round width (3 PSUM banks)
BANK_W = 512  # f32 columns per PSUM bank
PATW = 128  # mask pattern window width
NEG = -1.0e30

PS_BUFS = 2
PO_BUFS = 2
ES_BUFS = 6
ACC_BUFS = 6
CHUNK_ORDER = "byJ"  # "byJ" or "byH"
# For_i(staggered_reset=True) drops the per-iteration all-engine barrier
# + semaphore-reset block (~6.6 us/rep measured) in the reps-loop used
# for timing; the body is split into 4 semaphore stages instead.
STAGGERED = False  # measured: staggered stage transitions cost more
                   # than the single barrier they replace (92.6 vs 90.4 us)
# Hybrid exp: every DVE_EVERY-th all-full round computes exp on DVE via a
# one-instruction Schraudolph approximation (y = bitcast_f16(int16(A*x+B)),
# ~3% max rel err on those tiles, round-to-nearest verified on HW) to
# offload the saturated ACT engine. 0 disables.
DVE_EVERY = 0
LOG2E = 1.4426950408889634
SCH_A = 1024.0 * LOG2E  # multiplied by SCALE at emission
SCH_B = 15360.0 - 44.4  # balanced two-sided error

NJ = S // CHUNK  # q chunks
NK = S // KT  # k tiles

_program_cache = {}
last_exec_time_ns = None
last_results = None


def _schedule_from_mask(bm):
    """Classify each (q-chunk J, k-tile j) as full / empty / partial and
    pack each chunk's tiles into exp rounds.

    Returns (cache_key, sched, patterns): sched[J] is a list of rounds,
    each round a (tiles, used) pair with tiles = [(j, q0, pat_idx, off)].
    patterns is a list of [KT, PATW] f32 additive-mask windows (0 where
    attending, NEG where masked), k-major. Partial tiles must have all
    cells active outside the window rows [q0, q0+PATW) (holds for the
    diffusion block-causal mask).
    """
    patterns = []
    pat_rects = []
    pat_idx = {}
    per_J = []  # per q-chunk: ordered tile list [(j, q0, pat_idx)]
    for J in range(NJ):
        rows = bm[J * CHUNK : (J + 1) * CHUNK]  # [CHUNK q, S k]
        fulls = []
        parts = []
        for j in range(NK):
            sub = rows[:, j * KT : (j + 1) * KT]  # [q, k]
            if sub.all():
                fulls.append((j, 0, None))
            elif not sub.any():
                continue
            else:
                q0 = int(np.argmax(sub.any(axis=1)))
                w = CHUNK - q0
                pw = min(PATW, w)
                if q0 + pw < CHUNK:
                    assert sub[q0 + pw :].all(), (
                        "mask cells outside the 128-row window are not all "
                        "active; unsupported mask structure"
                    )
                win = sub[q0 : q0 + pw]  # [pw, KT]
                key = win.tobytes()
                if key not in pat_idx:
                    pat_idx[key] = len(patterns)
                    pat = np.zeros((KT, PATW), np.float32)
                    pat[:, :pw] = np.where(
                        win.T, np.float32(0.0), np.float32(NEG)
                    )
                    # Masked cells as per-column k-suffix rectangles (cols
                    # grouped by equal suffix start): lets the kernel zero
                    # the es staircase with a few Pool-engine memsets
                    # instead of a mask-add matmul on PE. None if the
                    # masked set is not suffix-form (fallback: matmul).
                    rects = []
                    ok = True
                    winT = win.T  # [KT, pw] k-major
                    starts = np.full(pw, KT, np.int64)
                    for c in range(pw):
                        col = winT[:, c]
                        n_act = int(col.sum())
                        if not col[:n_act].all():
                            ok = False
                            break
                        starts[c] = n_act
                    if ok:
                        c = 0
                        while c < pw:
                            c2 = c
                            while c2 < pw and starts[c2] == starts[c]:
                                c2 += 1
                            if starts[c] < KT:
                                rects.append((c, c2, int(starts[c])))
                            c = c2
                    patterns.append(pat)
                    pat_rects.append(rects if ok else None)
                parts.append((j, q0, pat_idx[key]))
        assert fulls or parts, f"q-chunk {J} attends to nothing"
        parts.sort(key=lambda t: t[1])  # widest first
        tiles = fulls + parts
        assert tiles[0][1] == 0, "chunk needs a q0 == 0 tile first"
        per_J.append(tiles)

    # Global packing: flatten all (h, J) chunks (by-J phases) into one
    # tile stream and first-fit into uniform ROUND_W rounds such that no
    # tile crosses a PSUM bank and no gaps form (gap cells would be
    # exp'd stale PSUM). A lookahead of one chunk fills bank remainders
    # at chunk boundaries. A chunk's first placed tile must be its
    # q0 == 0 tile (PV/acc accumulation start covers the full q range).
    order = [(h, J) for J in sorted(range(NJ), reverse=True) for h in range(G)]
    queues = [
        [(h, J, j, q0, p) for (j, q0, p) in per_J[J]] for h, J in order
    ]
    rounds = []  # [( [(h,J,j,q0,pidx,off)...], used )]
    live = []  # queue indices started & unfinished (max 2: po banks)
    nexti = 0
    cur = []
    off = 0
    while live or nexti < len(queues):
        rem = min(ROUND_W - off, BANK_W - (off % BANK_W))
        cands = [
            (qi, t) for qi in live for t in queues[qi] if CHUNK - t[3] <= rem
        ]
        if len(live) < 2 and nexti < len(queues):
            t0 = queues[nexti][0]  # a chunk opens with its q0==0 tile
            if CHUNK - t0[3] <= rem:
                cands.append((nexti, t0))
        if not cands:
            assert cur, "packing deadlock"
            rounds.append((cur, off))
            cur = []
            off = 0
            continue
        # widest first; tie-break toward the oldest chunk (drain early)
        qi, t = min(cands, key=lambda c: (-(CHUNK - c[1][3]), c[0]))
        h, J, j, q0, p = t
        queues[qi].remove(t)
        if qi == nexti:
            live.append(qi)
            nexti += 1
        if not queues[qi]:
            live.remove(qi)
        cur.append((h, J, j, q0, p, off))
        off += CHUNK - q0
        if not rounds and len(cur) == 1:
            # Keep the FIRST round a single tile: after each reps-loop
            # barrier the P-state-throttled PE computes one cold QK
            # matmul instead of three before the first exp can start
            # (~0.9 us/iteration). Total active width is 512 + 45*1536
            # exactly, so the call count stays at 46.
            rounds.append((cur, off))
            cur = []
            off = 0
    if cur:
        rounds.append((cur, off))
    ntiles = {}
    for tiles, _ in rounds:
        for h, J, j, q0, p, off in tiles:
            ntiles[(h, J)] = ntiles.get((h, J), 0) + 1
    sched = (rounds, ntiles, pat_rects)
    cache_key = (
        tuple(tuple(tuple(t) for t in r) + (u,) for r, u in rounds),
        tuple(p.tobytes() for p in patterns),
        tuple(tuple(r) if r is not None else None for r in pat_rects),
    )
    return hash(cache_key), sched, patterns


def _build_program(sched, patterns, reps=1, unroll=1):
    import contextlib

    import concourse.bacc as bacc
    import concourse.tile as tile
    from concourse import mybir

    f32 = mybir.dt.float32
    f16 = mybir.dt.float16
    bf16 = mybir.dt.bfloat16
    EXP = mybir.ActivationFunctionType.Exp

    nc = bacc.Bacc(
        "TRN2", target_bir_lowering=False, debug=False, num_devices=NCORES
    )

    qT = nc.dram_tensor("qT", [G, D, S], f16, kind="ExternalInput").ap()
    kT = nc.dram_tensor("kT", [D, S], f16, kind="ExternalInput").ap()
    v = nc.dram_tensor("v", [S, D], f16, kind="ExternalInput").ap()
    n_pat = max(1, len(patterns))
    pmask = nc.dram_tensor(
        "pmask", [n_pat, KT, PATW], bf16, kind="ExternalInput"
    ).ap()
    ident = nc.dram_tensor("ident", [D, D], bf16, kind="ExternalInput").ap()
    # Merged per-chunk epilogue output: [:, :CHUNK] = denominator
    # partial sums (k-partition rows), [:, CHUNK:] = O^T chunk (d rows).
    # One DMA per chunk instead of two (fewer issue slots + completion
    # semaphores; shorter drain on the final chunk).
    ep_d = nc.dram_tensor(
        "ep_d", [G * NJ, KT, 2 * CHUNK], f16, kind="ExternalOutput"
    ).ap()

    with tile.TileContext(nc) as tc:
        with (
            tc.tile_pool(name="singles", bufs=1) as singles,
            tc.tile_pool(name="ps", bufs=PS_BUFS, space="PSUM") as ps_pool,
            tc.tile_pool(name="po", bufs=PO_BUFS, space="PSUM") as po_pool,
            tc.tile_pool(name="es", bufs=ES_BUFS) as es_pool,
            tc.tile_pool(name="accp", bufs=ACC_BUFS) as acc_pool,
        ):
            qT_sb = singles.tile([D, G * S], f16)
            kT_sb = singles.tile([D, S], f16)
            v_sb = singles.tile([KT, NK * D], f16)
            pm_sb = singles.tile([KT, n_pat * PATW], bf16)
            id_sb = singles.tile([D, D], bf16)
            dummy = singles.tile([1, 1], f32)

            # Input DMAs, ordered for the startup critical path. Chunk
            # order is by-J phases (h0..h3 at J3, then J2, J1, J0), so
            # load kT's first tiles + everyone's J3 q-slices first, then
            # the bulk, then the later q phases.
            # The two DMAs gating the first QK round issue on different
            # HWDGE queues (SP + Activation) so their ~0.65 us issue
            # latencies overlap; ACT is otherwise idle at startup.
            nc.sync.dma_start(out=kT_sb[:, 0:CHUNK], in_=kT[:, 0:CHUNK])
            nc.scalar.dma_start(
                out=qT_sb[:, 3 * CHUNK : 4 * CHUNK],
                in_=qT[0][:, 3 * CHUNK : 4 * CHUNK],
            )
            nc.sync.dma_start(out=kT_sb[:, CHUNK:], in_=kT[:, CHUNK:])
            nc.sync.dma_start(
                out=qT_sb[:, S:].rearrange(
                    "p (h s) -> p h s", s=S
                )[:, :, 3 * CHUNK : 4 * CHUNK],
                in_=qT[1:].rearrange("h p s -> p h s")[
                    :, :, 3 * CHUNK : 4 * CHUNK
                ],
            )
            nc.sync.dma_start(
                out=v_sb.rearrange("p (t d) -> p t d", d=D),
                in_=v.rearrange("(t p) d -> p t d", p=KT),
            )
            nc.sync.dma_start(
                out=pm_sb.rearrange("p (n c) -> p n c", c=PATW),
                in_=pmask.rearrange("n p c -> p n c"),
            )
            nc.sync.dma_start(out=id_sb, in_=ident)
            for Jc in (2, 1, 0):
                nc.sync.dma_start(
                    out=qT_sb.rearrange("p (h s) -> p h s", s=S)[
                        :, :, Jc * CHUNK : (Jc + 1) * CHUNK
                    ],
                    in_=qT.rearrange("h p s -> p h s")[
                        :, :, Jc * CHUNK : (Jc + 1) * CHUNK
                    ],
                )

            # Hoist the activation-table load out of the reps loop.
            nc.vector.memset(dummy, 0.0)
            nc.scalar.activation(dummy, dummy, EXP, scale=1.0)

            rep_ctx = (
                tc.For_i(0, reps, 1, staggered_reset=STAGGERED)
                if reps > 1
                else contextlib.nullcontext()
            )

            rounds, ntiles, pat_rects = sched

            def emit_pv(prev, ctxs, final=False):
                tiles, es, used = prev
                for h, J, j, q0, pidx, off in tiles:
                    ctx = ctxs[(h, J)]
                    po = ctx["po"]
                    w = CHUNK - q0
                    sl = es[:, off : off + w]
                    first = ctx["done"] == 0
                    last = ctx["done"] == ctx["ntiles"] - 1
                    nc.tensor.matmul(
                        po[:, q0:],
                        lhsT=v_sb[:, j * D : (j + 1) * D],
                        rhs=sl,
                        start=first,
                        stop=last,
                    )
                    # Denominator partial sums on DVE (fp16 2x mode) into
                    # the left half of the merged epilogue tile; the right
                    # half receives the O^T copy. Host splits and reduces.
                    acc = ctx["ep"][:, :CHUNK]
                    if first:
                        nc.vector.tensor_copy(acc, sl)
                    else:
                        nc.vector.tensor_add(acc[:, q0:], acc[:, q0:], sl)
                    ctx["done"] += 1
                    if ctx["done"] == ctx["ntiles"]:
                        ci = h * NJ + J
                        if final:
                            # Drain only: ACT is idle after the last exp,
                            # so its Copy runs in parallel with DVE's
                            # final accumulator adds.
                            nc.scalar.copy(ctx["ep"][:, CHUNK:], po)
                        else:
                            nc.vector.tensor_copy(ctx["ep"][:, CHUNK:], po)
                        nc.sync.dma_start(out=ep_d[ci], in_=ctx["ep"])
                        del ctxs[(h, J)]

            eligible = [
                ri
                for ri, (tiles, _) in enumerate(rounds)
                if all(t[4] is None for t in tiles)
            ]
            dve_rounds = (
                set(eligible[DVE_EVERY - 1 :: DVE_EVERY]) if DVE_EVERY else set()
            )
            n_rounds = len(rounds)
            bounds = {
                (n_rounds * (s + 1)) // 4 for s in range(3)
            } if (reps > 1 and STAGGERED) else set()

            with rep_ctx:
                for _ in range(unroll):
                    ctxs = {}
                    prev = None  # (tiles, es, used) awaiting PV emission
                    for ri, (tiles, used) in enumerate(rounds):
                        if ri in bounds:
                            tc.stage_boundary()
                        ps = ps_pool.tile(
                            [KT, ROUND_W], f32, tag="ps", name="ps"
                        )
                        for h, J, j, q0, pidx, off in tiles:
                            if (h, J) not in ctxs:
                                ctxs[(h, J)] = {
                                    "po": po_pool.tile(
                                        [D, CHUNK], f32, tag="po", name="po"
                                    ),
                                    "ep": acc_pool.tile(
                                        [KT, 2 * CHUNK], f16, tag="ep",
                                        name="ep",
                                    ),
                                    "done": 0,
                                    "ntiles": ntiles[(h, J)],
                                }
                            w = CHUNK - q0
                            rhs_q = qT_sb[
                                :,
                                h * S + J * CHUNK + q0 : h * S
                                + (J + 1) * CHUNK,
                            ]
                            nc.tensor.matmul(
                                ps[:, off : off + w],
                                lhsT=kT_sb[:, j * KT : (j + 1) * KT],
                                rhs=rhs_q,
                                start=True,
                                stop=(pidx is None),
                            )
                            if pidx is not None:
                                pw = min(PATW, w)
                                nc.tensor.matmul(
                                    ps[:, off : off + pw],
                                    lhsT=id_sb,
                                    rhs=pm_sb[
                                        :, pidx * PATW : pidx * PATW + pw
                                    ],
                                    start=False,
                                    stop=True,
                                )
                        if prev is not None:
                            emit_pv(prev, ctxs)
                            prev = None
                        es = es_pool.tile(
                            [KT, ROUND_W], f16, tag="es", name="es"
                        )
                        if ri in dve_rounds:
                            nc.vector.tensor_scalar(
                                es[:, :used].bitcast(mybir.dt.int16),
                                ps[:, :used],
                                SCH_A * SCALE,
                                SCH_B,
                                mybir.AluOpType.mult,
                                mybir.AluOpType.add,
                            )
                        else:
                            nc.scalar.activation(
                                es[:, :used], ps[:, :used], EXP, scale=SCALE
                            )
                        prev = (tiles, es, used)
                    emit_pv(prev, ctxs, final=True)
                    prev = None

    # Pin the ACT table set to the one containing Exp so the table-load
    # pass emits exactly one load (hoisted to the pre-loop dummy exp).
    import concourse.bacc as bacc_mod

    orig_tables = bacc_mod.get_activation_tables

    def _only_ln_exp_set(arch):
        return {
            name: (fns if name == "natural_log_exp_and_others" else set())
            for name, fns in orig_tables(arch).items()
        }

    bacc_mod.get_activation_tables = _only_ln_exp_set
    try:
        nc.compile()
    finally:
        bacc_mod.get_activation_tables = orig_tables
    return nc


def _get_program(bm):
    key, sched, patterns = _schedule_from_mask(bm)
    if key not in _program_cache:
        _program_cache[key] = _build_program(sched, patterns)
    return _program_cache[key], patterns


def _shard_inputs(q, k, v, patterns):
    import ml_dtypes

    bf16 = ml_dtypes.bfloat16
    n_pat = max(1, len(patterns))
    if patterns:
        pm = np.ascontiguousarray(np.stack(patterns).astype(bf16))
    else:
        pm = np.zeros((n_pat, KT, PATW), bf16)
    ident = np.eye(D, dtype=bf16)

    q5 = q.reshape(S, HKV, G, D)
    k4 = k.reshape(S, HKV, D)
    v4 = v.reshape(S, HKV, D)
    in_maps = []
    for c in range(NCORES):
        qTc = np.ascontiguousarray(
            q5[:, c].transpose(1, 2, 0).astype(np.float16)
        )  # [G, D, S]
        kTc = np.ascontiguousarray(k4[:, c].T.astype(np.float16))  # [D, S]
        vc = np.ascontiguousarray(v4[:, c].astype(np.float16))  # [S, D]
        in_maps.append(
            {
                "qT": qTc,
                "kT": kTc,
                "v": vc,
                "pmask": pm,
                "ident": ident,
            }
        )
    return in_maps


def kernel(q, k, v, block_mask):
    global last_exec_time_ns, last_results
    q = np.ascontiguousarray(np.asarray(q, dtype=np.float32))
    k = np.ascontiguousarray(np.asarray(k, dtype=np.float32))
    v = np.ascontiguousarray(np.asarray(v, dtype=np.float32))
    bm = np.ascontiguousarray(np.asarray(block_mask)).astype(bool)

    nc, patterns = _get_program(bm)
    in_maps = _shard_inputs(q, k, v, patterns)

    from concourse.bass_utils import run_bass_kernel_spmd

    res = run_bass_kernel_spmd(nc, in_maps, list(range(NCORES)), trace=False)
    last_exec_time_ns = res.exec_time_ns
    last_results = res

    out = np.empty((S, H * D), np.float32)
    for c in range(NCORES):
        ep = res.results[c]["ep_d"]  # [G*NJ, KT, 2*CHUNK] f16
        l = (
            ep[:, :, :CHUNK].astype(np.float32).sum(axis=1)
        ).reshape(G, NJ * CHUNK)  # [G, S]
        oTc = (
            ep[:, :, CHUNK:]
            .astype(np.float32)
            .reshape(G, NJ, D, CHUNK)
            .transpose(0, 2, 1, 3)
            .reshape(G, D, S)
        )
        oTc = oTc / l[:, None, :]
        out[:, c * G * D : (c + 1) * G * D] = (
            oTc.transpose(2, 0, 1).reshape(S, G * D)
        )
    return out
